# revision 29
# baseline (speedup 1.0000x reference)
"""DistanceAndAngle kernel for 8 Trainium2 NeuronCores.

Strategy (edge/triplet data-parallel, per the sharding hint):
  - Shard edges and triplets contiguously across the 8 cores.
  - Host-side staging is limited to index-driven *layout* of input data
    (sharding/replication/gathers of existing values + dtype casts); every
    FLOP of the reference computation runs on the NeuronCores:
      launch 1 (edge phase):  shift_vec = einsum(shift, lattice[batch[i]]),
                              pair = pos[j] + shift_vec - pos[i],
                              dist = sqrt(sum(pair^2))  -> distances,
                              unit-vector planes u = pair * (1/dist)
      launch 2 (triplet phase): cos = u1 . u2 (identical to dot/(r1*r2)),
                              clip, NaN where a zero-length edge is involved
                              (u = 0*inf = NaN there) -> angles
  - All tensors are staged in SoA plane layout ([n_planes, M]) so every DVE
    op is contiguous (strided 4-slot ops run at ~1.2-2.6 cyc/elem vs ~0.64
    contiguous); one 3D-AP DMA loads all planes of a tile at once.
  - The rec planes produced by launch 1 are re-sharded on the host by the
    triplet edge indices (pure data movement) and streamed in launch 2.

On-device random-access DMA on this runtime tops out at ~128 rows / 1.4us
(single-index-per-partition SWDGE indirect DMA; the vector_dynamic_offsets
DGE level is disabled), i.e. ~50ms+ for the 4.8M random rows this problem
needs per core -- 250x off the memory roofline.  Streaming the staged data
instead runs the whole problem near the DMA/DVE roofline.
"""

import numpy as np

NCORES = 8
E = 3_200_000
T = 16_000_000
N = 100_000
ES = E // NCORES          # 400_000 edges per core
TS = T // NCORES          # 2_000_000 triplets per core
KE = 625                  # edge-tile free dim:    ES = 128 * KE * NTE
NTE = ES // (128 * KE)    # 5 edge tiles
KT = 625                  # triplet-tile free dim: TS = 128 * KT * NTT
NTT = TS // (128 * KT)    # 25 triplet tiles
PCH = 65535 // KE         # partition chunk for DRAM-dest stores (16-bit field)

TRACE = False             # test harness sets kernel.TRACE = True for profiling
last_info = {}            # exec times / profiling info from the latest call

_cache = {}


def _install_ntff_hook():
    import sys
    import types
    try:
        from antenv.axon_hooks import get_axon_ntff_profile_hook  # noqa: F401
        return
    except ImportError:
        pass
    try:
        from trn_agent_boot.trn_boot import _ntff_profile_via_ctypes
    except ImportError:
        return
    hook = _ntff_profile_via_ctypes("/opt/axon/libaxon_pjrt.so")
    mod = types.ModuleType("antenv.axon_hooks")
    _h = {"hook": hook}
    mod.get_axon_ntff_profile_hook = lambda: _h["hook"]
    mod.set_axon_ntff_profile_hook = lambda h: _h.__setitem__("hook", h)
    sys.modules["antenv.axon_hooks"] = mod


def _build_edge_prog():
    import concourse.bacc as bacc
    import concourse.tile as tile
    import concourse.mybir as mybir

    nc = bacc.Bacc("TRN2", target_bir_lowering=False, debug=False, num_devices=NCORES)
    # SoA planes merged into two 9-plane tensors (one per HWDGE ring):
    # ea = [posi(3) | posj(3) | lat_row0(3)], eb = [lat_row1(3) | lat_row2(3) | shf(3)]
    ea_d = nc.dram_tensor("ea", [9, ES], mybir.dt.float32, kind="ExternalInput")
    eb_d = nc.dram_tensor("eb", [9, ES], mybir.dt.float32, kind="ExternalInput")
    recp_d = nc.dram_tensor("recp", [3, ES], mybir.dt.float32, kind="ExternalOutput")
    dist_d = nc.dram_tensor("dist", [ES], mybir.dt.float32, kind="ExternalOutput")

    M = 128 * KE  # edges per tile
    with tile.TileContext(nc) as tc:
        with tc.tile_pool(name="io", bufs=2) as io, \
             tc.tile_pool(name="wk", bufs=4) as wk:
            for t in range(NTE):
                r0 = t * M

                # split each 9-plane load by consumer order: pos planes and
                # shift land first so the DVE chain starts while the lattice
                # planes are still streaming.
                A = io.tile([128, 9, KE], mybir.dt.float32, tag="A", name="A")
                nc.sync.dma_start(out=A[:, 0:6, :],
                                  in_=ea_d[0:6, r0:r0 + M].rearrange("c (p k) -> p c k", p=128))
                B = io.tile([128, 9, KE], mybir.dt.float32, tag="B", name="B")
                nc.scalar.dma_start(out=B[:, 6:9, :],
                                    in_=eb_d[6:9, r0:r0 + M].rearrange("c (p k) -> p c k", p=128))
                nc.sync.dma_start(out=A[:, 6:9, :],
                                  in_=ea_d[6:9, r0:r0 + M].rearrange("c (p k) -> p c k", p=128))
                nc.scalar.dma_start(out=B[:, 0:6, :],
                                    in_=eb_d[0:6, r0:r0 + M].rearrange("c (p k) -> p c k", p=128))
                pi = A[:, 0:3, :]
                pj = A[:, 3:6, :]
                lts = [A[:, 6:9, :], B[:, 0:3, :], B[:, 3:6, :]]
                sf = B[:, 6:9, :]

                rec = wk.tile([128, 3, KE], mybir.dt.float32, tag="rec")
                u3 = wk.tile([128, 3, KE], mybir.dt.float32, tag="u3")
                # rec = posj - posi + sum_c shf_c * lat_row_c, computed as
                # wide [128, 3K] contiguous ops (shf_c broadcast over the
                # 3-plane middle dim) to amortize per-instruction overhead.
                rec3 = rec[:, :, :]
                nc.vector.tensor_sub(out=rec3, in0=pj, in1=pi)
                for c in range(3):
                    nc.vector.tensor_mul(out=u3[:], in0=sf[:, c:c + 1, :].to_broadcast([128, 3, KE]),
                                         in1=lts[c])
                    nc.vector.tensor_add(out=rec3, in0=rec3, in1=u3[:])
                # d2 = dx^2 + dy^2 + dz^2 ; dist = sqrt(d2) ; rec q-plane = 1/dist
                # (square runs on the otherwise-idle ACT engine)
                nc.scalar.activation(out=u3[:], in_=rec3,
                                     func=mybir.ActivationFunctionType.Square)
                d2 = wk.tile([128, KE], mybir.dt.float32, tag="d2")
                nc.vector.tensor_add(out=d2[:], in0=u3[:, 0, :], in1=u3[:, 1, :])
                nc.vector.tensor_add(out=d2[:], in0=d2[:], in1=u3[:, 2, :])
                dt = wk.tile([128, KE], mybir.dt.float32, tag="dt")
                nc.scalar.activation(out=dt[:], in_=d2[:],
                                     func=mybir.ActivationFunctionType.Sqrt)
                # normalize to unit vectors (1/0 = inf -> u = 0*inf = NaN for
                # zero-length edges, which propagates to the angle as in the
                # reference)
                qr = wk.tile([128, KE], mybir.dt.float32, tag="qr")
                nc.vector.reciprocal(out=qr[:], in_=dt[:])
                nc.vector.tensor_mul(out=rec3, in0=rec3,
                                     in1=qr[:, None, :].to_broadcast([128, 3, KE]))
                # stores: DRAM-dest merged dims must stay <= 65535 elems, so
                # chunk along partitions; [p, c, k] DRAM view matches the SBUF
                # iteration order.
                recp_view = recp_d[:, r0:r0 + M].rearrange("c (p k) -> p c k", p=128)
                dist_view = dist_d[r0:r0 + M].rearrange("(p k) -> p k", p=128)
                for q in range(0, 128, PCH):
                    qe = min(q + PCH, 128)
                    nc.gpsimd.dma_start(out=recp_view[q:qe], in_=rec[q:qe])
                    nc.gpsimd.dma_start(out=dist_view[q:qe], in_=dt[q:qe])
    nc.compile()
    return nc


def _build_tri_prog():
    import concourse.bacc as bacc
    import concourse.tile as tile
    import concourse.mybir as mybir

    nc = bacc.Bacc("TRN2", target_bir_lowering=False, debug=False, num_devices=NCORES)
    r1_d = nc.dram_tensor("r1", [3, TS], mybir.dt.float32, kind="ExternalInput")
    r2_d = nc.dram_tensor("r2", [3, TS], mybir.dt.float32, kind="ExternalInput")
    ang_d = nc.dram_tensor("ang", [TS], mybir.dt.float32, kind="ExternalOutput")

    import concourse.mybir as mb

    # tile plan: 12 tiles of KT2=1250 + 1 of 625 (15625 = 12*1250 + 625);
    # bigger tiles amortize the ~270ns fixed cost of each DVE instruction.
    KT2 = 1250
    tiles = [(i * 128 * KT2, KT2) for i in range(12)] + [(12 * 128 * KT2, 625)]

    with tile.TileContext(nc) as tc:
        with tc.tile_pool(name="io", bufs=3) as io, \
             tc.tile_pool(name="wk", bufs=3) as wk:
            for r0, kt in tiles:
                M = 128 * kt
                a = io.tile([128, 4, kt], mybir.dt.float32, tag="a", padded_shape=[128, 4, KT2])
                nc.sync.dma_start(out=a[:],
                                  in_=r1_d[:, r0:r0 + M].rearrange("c (p k) -> p c k", p=128))
                b = io.tile([128, 4, kt], mybir.dt.float32, tag="b", padded_shape=[128, 4, KT2])
                nc.scalar.dma_start(out=b[:],
                                    in_=r2_d[:, r0:r0 + M].rearrange("c (p k) -> p c k", p=128))

                dot = wk.tile([128, kt], mybir.dt.float32, tag="dot", padded_shape=[128, KT2])
                tt = wk.tile([128, kt], mybir.dt.float32, tag="tt", padded_shape=[128, KT2])
                qq = wk.tile([128, kt], mybir.dt.float32, tag="qq", padded_shape=[128, KT2])
                nc.vector.tensor_mul(out=dot[:], in0=a[:, 0, :], in1=b[:, 0, :])
                nc.vector.tensor_mul(out=tt[:], in0=a[:, 1, :], in1=b[:, 1, :])
                nc.vector.tensor_add(out=dot[:], in0=dot[:], in1=tt[:])
                nc.vector.tensor_mul(out=tt[:], in0=a[:, 2, :], in1=b[:, 2, :])
                nc.vector.tensor_add(out=dot[:], in0=dot[:], in1=tt[:])
                nc.vector.tensor_mul(out=qq[:], in0=a[:, 3, :], in1=b[:, 3, :])
                # cos reuses tt; clip to [-1, 1]; DVE max/min suppress NaN, so
                # re-poison where a zero-length edge is involved
                # (qq = inf -> inf*0 = NaN); ang reuses dot.
                nc.vector.tensor_mul(out=tt[:], in0=dot[:], in1=qq[:])
                nc.vector.tensor_scalar(out=tt[:], in0=tt[:], scalar1=-1.0, scalar2=1.0,
                                        op0=mb.AluOpType.max, op1=mb.AluOpType.min)
                nc.vector.scalar_tensor_tensor(out=dot[:], in0=qq[:], scalar=0.0, in1=tt[:],
                                               op0=mb.AluOpType.mult, op1=mb.AluOpType.add)
                ang_view = ang_d[r0:r0 + M].rearrange("(p k) -> p k", p=128)
                pch = 65535 // kt
                engs = [nc.sync, nc.scalar]
                for ci, q in enumerate(range(0, 128, pch)):
                    qe = min(q + pch, 128)
                    engs[ci % 2].dma_start(out=ang_view[q:qe], in_=dot[q:qe])
    nc.compile()
    return nc


def kernel(batch, lattice, pos, edge_index, edge_cell_shift, triplet_edge_index):
    import time
    from concourse.bass_utils import run_bass_kernel_spmd
    import concourse.bass_utils as bass_utils

    if TRACE:
        _install_ntff_hook()
        bass_utils.upload_artifacts = lambda tmpdir: tmpdir

    batch = np.asarray(batch)
    lattice = np.asarray(lattice, dtype=np.float32)
    pos = np.asarray(pos, dtype=np.float32)
    edge_index = np.asarray(edge_index)
    edge_cell_shift = np.asarray(edge_cell_shift)
    triplet_edge_index = np.asarray(triplet_edge_index)

    # ---- host staging for the edge launch: pure index-driven layout (SoA) ----
    ei0 = edge_index[0]
    ei1 = edge_index[1]
    posT = np.ascontiguousarray(pos.T)               # [3, N]
    batch_e = batch[ei0]
    latP = np.ascontiguousarray(lattice.transpose(1, 2, 0))  # [c, d, 64]
    ea = np.empty((9, E), dtype=np.float32)
    ea[0:3] = posT[:, ei0]
    ea[3:6] = posT[:, ei1]
    ea[6:9] = latP[0][:, batch_e]
    eb = np.empty((9, E), dtype=np.float32)
    eb[0:3] = latP[1][:, batch_e]
    eb[3:6] = latP[2][:, batch_e]
    eb[6:9] = edge_cell_shift.T

    if "edge" not in _cache:
        _cache["edge"] = _build_edge_prog()
    nc_edge = _cache["edge"]

    in_maps = []
    for c in range(NCORES):
        s = slice(c * ES, (c + 1) * ES)
        in_maps.append({
            "ea": np.ascontiguousarray(ea[:, s]),
            "eb": np.ascontiguousarray(eb[:, s]),
        })

    t0 = time.time()
    res1 = run_bass_kernel_spmd(nc_edge, in_maps, list(range(NCORES)), trace=TRACE)
    last_info["edge_wall_s"] = time.time() - t0
    last_info["edge_exec_ns"] = res1.exec_time_ns

    distances = np.concatenate([res1.results[c]["dist"] for c in range(NCORES)])
    recp = np.concatenate([res1.results[c]["recp"] for c in range(NCORES)], axis=1)  # [4, E]

    # ---- host staging for the triplet launch: re-shard rec planes ----
    r1 = recp[:, triplet_edge_index[0]]              # [4, T] f32
    r2 = recp[:, triplet_edge_index[1]]              # [4, T] f32

    if "tri" not in _cache:
        _cache["tri"] = _build_tri_prog()
    nc_tri = _cache["tri"]

    in_maps2 = []
    for c in range(NCORES):
        s = slice(c * TS, (c + 1) * TS)
        in_maps2.append({
            "r1": np.ascontiguousarray(r1[:, s]),
            "r2": np.ascontiguousarray(r2[:, s]),
        })

    t0 = time.time()
    res2 = run_bass_kernel_spmd(nc_tri, in_maps2, list(range(NCORES)), trace=TRACE)
    last_info["tri_wall_s"] = time.time() - t0
    last_info["tri_exec_ns"] = res2.exec_time_ns

    angles = np.concatenate([res2.results[c]["ang"] for c in range(NCORES)])
    return distances, angles


# revision 31
# speedup vs baseline: 1.1154x; 1.1154x over previous
"""DistanceAndAngle kernel for 8 Trainium2 NeuronCores.

Strategy (edge/triplet data-parallel, per the sharding hint):
  - Shard edges and triplets contiguously across the 8 cores.
  - Host-side staging is limited to index-driven *layout* of input data
    (sharding/replication/gathers of existing values + dtype casts); every
    FLOP of the reference computation runs on the NeuronCores:
      launch 1 (edge phase):  shift_vec = einsum(shift, lattice[batch[i]]),
                              pair = pos[j] + shift_vec - pos[i],
                              dist = sqrt(sum(pair^2))  -> distances,
                              unit-vector planes u = pair * (1/dist)
      launch 2 (triplet phase): cos = u1 . u2 (identical to dot/(r1*r2)),
                              clip, NaN where a zero-length edge is involved
                              (u = 0*inf = NaN there) -> angles
  - All tensors are staged in SoA plane layout ([n_planes, M]) so every DVE
    op is contiguous (strided 4-slot ops run at ~1.2-2.6 cyc/elem vs ~0.64
    contiguous); one 3D-AP DMA loads all planes of a tile at once.
  - The rec planes produced by launch 1 are re-sharded on the host by the
    triplet edge indices (pure data movement) and streamed in launch 2.

On-device random-access DMA on this runtime tops out at ~128 rows / 1.4us
(single-index-per-partition SWDGE indirect DMA; the vector_dynamic_offsets
DGE level is disabled), i.e. ~50ms+ for the 4.8M random rows this problem
needs per core -- 250x off the memory roofline.  Streaming the staged data
instead runs the whole problem near the DMA/DVE roofline.
"""

import numpy as np

NCORES = 8
E = 3_200_000
T = 16_000_000
N = 100_000
ES = E // NCORES          # 400_000 edges per core
TS = T // NCORES          # 2_000_000 triplets per core
KE = 625                  # edge-tile free dim:    ES = 128 * KE * NTE
NTE = ES // (128 * KE)    # 5 edge tiles
KT = 625                  # triplet-tile free dim: TS = 128 * KT * NTT
NTT = TS // (128 * KT)    # 25 triplet tiles
PCH = 65535 // KE         # partition chunk for DRAM-dest stores (16-bit field)

TRACE = False             # test harness sets kernel.TRACE = True for profiling
last_info = {}            # exec times / profiling info from the latest call

_cache = {}


def _install_ntff_hook():
    import sys
    import types
    try:
        from antenv.axon_hooks import get_axon_ntff_profile_hook  # noqa: F401
        return
    except ImportError:
        pass
    try:
        from trn_agent_boot.trn_boot import _ntff_profile_via_ctypes
    except ImportError:
        return
    hook = _ntff_profile_via_ctypes("/opt/axon/libaxon_pjrt.so")
    mod = types.ModuleType("antenv.axon_hooks")
    _h = {"hook": hook}
    mod.get_axon_ntff_profile_hook = lambda: _h["hook"]
    mod.set_axon_ntff_profile_hook = lambda h: _h.__setitem__("hook", h)
    sys.modules["antenv.axon_hooks"] = mod


def _build_edge_prog():
    import concourse.bacc as bacc
    import concourse.tile as tile
    import concourse.mybir as mybir

    nc = bacc.Bacc("TRN2", target_bir_lowering=False, debug=False, num_devices=NCORES)
    # SoA planes merged into two 9-plane tensors (one per HWDGE ring):
    # ea = [posi(3) | posj(3) | lat_row0(3)], eb = [lat_row1(3) | lat_row2(3) | shf(3)]
    ea_d = nc.dram_tensor("ea", [9, ES], mybir.dt.float32, kind="ExternalInput")
    eb_d = nc.dram_tensor("eb", [9, ES], mybir.dt.float32, kind="ExternalInput")
    recp_d = nc.dram_tensor("recp", [3, ES], mybir.dt.float32, kind="ExternalOutput")
    dist_d = nc.dram_tensor("dist", [ES], mybir.dt.float32, kind="ExternalOutput")

    M = 128 * KE  # edges per tile
    with tile.TileContext(nc) as tc:
        with tc.tile_pool(name="io", bufs=2) as io, \
             tc.tile_pool(name="wk", bufs=4) as wk:
            for t in range(NTE):
                r0 = t * M

                # split each 9-plane load by consumer order: pos planes and
                # shift land first so the DVE chain starts while the lattice
                # planes are still streaming.
                A = io.tile([128, 9, KE], mybir.dt.float32, tag="A", name="A")
                nc.sync.dma_start(out=A[:, 0:6, :],
                                  in_=ea_d[0:6, r0:r0 + M].rearrange("c (p k) -> p c k", p=128))
                B = io.tile([128, 9, KE], mybir.dt.float32, tag="B", name="B")
                nc.scalar.dma_start(out=B[:, 6:9, :],
                                    in_=eb_d[6:9, r0:r0 + M].rearrange("c (p k) -> p c k", p=128))
                nc.sync.dma_start(out=A[:, 6:9, :],
                                  in_=ea_d[6:9, r0:r0 + M].rearrange("c (p k) -> p c k", p=128))
                nc.scalar.dma_start(out=B[:, 0:6, :],
                                    in_=eb_d[0:6, r0:r0 + M].rearrange("c (p k) -> p c k", p=128))
                pi = A[:, 0:3, :]
                pj = A[:, 3:6, :]
                lts = [A[:, 6:9, :], B[:, 0:3, :], B[:, 3:6, :]]
                sf = B[:, 6:9, :]

                rec = wk.tile([128, 3, KE], mybir.dt.float32, tag="rec")
                u3 = wk.tile([128, 3, KE], mybir.dt.float32, tag="u3")
                # rec = posj - posi + sum_c shf_c * lat_row_c, computed as
                # wide [128, 3K] contiguous ops (shf_c broadcast over the
                # 3-plane middle dim) to amortize per-instruction overhead.
                rec3 = rec[:, :, :]
                nc.vector.tensor_sub(out=rec3, in0=pj, in1=pi)
                for c in range(3):
                    nc.vector.tensor_mul(out=u3[:], in0=sf[:, c:c + 1, :].to_broadcast([128, 3, KE]),
                                         in1=lts[c])
                    nc.vector.tensor_add(out=rec3, in0=rec3, in1=u3[:])
                # d2 = dx^2 + dy^2 + dz^2 ; dist = sqrt(d2) ; rec q-plane = 1/dist
                # (square runs on the otherwise-idle ACT engine)
                nc.scalar.activation(out=u3[:], in_=rec3,
                                     func=mybir.ActivationFunctionType.Square)
                d2 = wk.tile([128, KE], mybir.dt.float32, tag="d2")
                nc.vector.tensor_add(out=d2[:], in0=u3[:, 0, :], in1=u3[:, 1, :])
                nc.vector.tensor_add(out=d2[:], in0=d2[:], in1=u3[:, 2, :])
                dt = wk.tile([128, KE], mybir.dt.float32, tag="dt")
                nc.scalar.activation(out=dt[:], in_=d2[:],
                                     func=mybir.ActivationFunctionType.Sqrt)
                # normalize to unit vectors. approx-reciprocal's BITWISE_NOT
                # seed maps +0.0 -> 0xFFFFFFFF = NaN and Newton keeps it, so
                # zero-length edges still yield non-finite u -> NaN angles as
                # in the reference. ~2.8x cheaper than the 8-slice reciprocal.
                qr = wk.tile([128, KE], mybir.dt.float32, tag="qr")
                nc.vector.reciprocal_approx_accurate(out=qr[:], in_=dt[:], scratch=d2[:])
                nc.vector.tensor_mul(out=rec3, in0=rec3,
                                     in1=qr[:, None, :].to_broadcast([128, 3, KE]))
                # stores: DRAM-dest merged dims must stay <= 65535 elems, so
                # chunk along partitions; [p, c, k] DRAM view matches the SBUF
                # iteration order.
                recp_view = recp_d[:, r0:r0 + M].rearrange("c (p k) -> p c k", p=128)
                dist_view = dist_d[r0:r0 + M].rearrange("(p k) -> p k", p=128)
                engs = [nc.sync, nc.scalar]
                for ci, q in enumerate(range(0, 128, PCH)):
                    qe = min(q + PCH, 128)
                    engs[ci % 2].dma_start(out=recp_view[q:qe], in_=rec[q:qe])
                    nc.scalar.dma_start(out=dist_view[q:qe], in_=dt[q:qe])
    nc.compile()
    return nc


def _build_tri_prog():
    import concourse.bacc as bacc
    import concourse.tile as tile
    import concourse.mybir as mybir

    nc = bacc.Bacc("TRN2", target_bir_lowering=False, debug=False, num_devices=NCORES)
    r1_d = nc.dram_tensor("r1", [3, TS], mybir.dt.float32, kind="ExternalInput")
    r2_d = nc.dram_tensor("r2", [3, TS], mybir.dt.float32, kind="ExternalInput")
    ang_d = nc.dram_tensor("ang", [TS], mybir.dt.float32, kind="ExternalOutput")

    import concourse.mybir as mb

    # tile plan: 12 tiles of KT2=1250 + 1 of 625 (15625 = 12*1250 + 625);
    # bigger tiles amortize the ~270ns fixed cost of each DVE instruction.
    KT2 = 1250
    tiles = [(i * 128 * KT2, KT2) for i in range(12)] + [(12 * 128 * KT2, 625)]

    with tile.TileContext(nc) as tc:
        with tc.tile_pool(name="io", bufs=3) as io, \
             tc.tile_pool(name="wk", bufs=3) as wk:
            for r0, kt in tiles:
                M = 128 * kt
                a = io.tile([128, 4, kt], mybir.dt.float32, tag="a", padded_shape=[128, 4, KT2])
                nc.sync.dma_start(out=a[:],
                                  in_=r1_d[:, r0:r0 + M].rearrange("c (p k) -> p c k", p=128))
                b = io.tile([128, 4, kt], mybir.dt.float32, tag="b", padded_shape=[128, 4, KT2])
                nc.scalar.dma_start(out=b[:],
                                    in_=r2_d[:, r0:r0 + M].rearrange("c (p k) -> p c k", p=128))

                dot = wk.tile([128, kt], mybir.dt.float32, tag="dot", padded_shape=[128, KT2])
                tt = wk.tile([128, kt], mybir.dt.float32, tag="tt", padded_shape=[128, KT2])
                qq = wk.tile([128, kt], mybir.dt.float32, tag="qq", padded_shape=[128, KT2])
                nc.vector.tensor_mul(out=dot[:], in0=a[:, 0, :], in1=b[:, 0, :])
                nc.vector.tensor_mul(out=tt[:], in0=a[:, 1, :], in1=b[:, 1, :])
                nc.vector.tensor_add(out=dot[:], in0=dot[:], in1=tt[:])
                nc.vector.tensor_mul(out=tt[:], in0=a[:, 2, :], in1=b[:, 2, :])
                nc.vector.tensor_add(out=dot[:], in0=dot[:], in1=tt[:])
                nc.vector.tensor_mul(out=qq[:], in0=a[:, 3, :], in1=b[:, 3, :])
                # cos reuses tt; clip to [-1, 1]; DVE max/min suppress NaN, so
                # re-poison where a zero-length edge is involved
                # (qq = inf -> inf*0 = NaN); ang reuses dot.
                nc.vector.tensor_mul(out=tt[:], in0=dot[:], in1=qq[:])
                nc.vector.tensor_scalar(out=tt[:], in0=tt[:], scalar1=-1.0, scalar2=1.0,
                                        op0=mb.AluOpType.max, op1=mb.AluOpType.min)
                nc.vector.scalar_tensor_tensor(out=dot[:], in0=qq[:], scalar=0.0, in1=tt[:],
                                               op0=mb.AluOpType.mult, op1=mb.AluOpType.add)
                ang_view = ang_d[r0:r0 + M].rearrange("(p k) -> p k", p=128)
                pch = 65535 // kt
                engs = [nc.sync, nc.scalar]
                for ci, q in enumerate(range(0, 128, pch)):
                    qe = min(q + pch, 128)
                    engs[ci % 2].dma_start(out=ang_view[q:qe], in_=dot[q:qe])
    nc.compile()
    return nc


def kernel(batch, lattice, pos, edge_index, edge_cell_shift, triplet_edge_index):
    import time
    from concourse.bass_utils import run_bass_kernel_spmd
    import concourse.bass_utils as bass_utils

    if TRACE:
        _install_ntff_hook()
        bass_utils.upload_artifacts = lambda tmpdir: tmpdir

    batch = np.asarray(batch)
    lattice = np.asarray(lattice, dtype=np.float32)
    pos = np.asarray(pos, dtype=np.float32)
    edge_index = np.asarray(edge_index)
    edge_cell_shift = np.asarray(edge_cell_shift)
    triplet_edge_index = np.asarray(triplet_edge_index)

    # ---- host staging for the edge launch: pure index-driven layout (SoA) ----
    ei0 = edge_index[0]
    ei1 = edge_index[1]
    posT = np.ascontiguousarray(pos.T)               # [3, N]
    batch_e = batch[ei0]
    latP = np.ascontiguousarray(lattice.transpose(1, 2, 0))  # [c, d, 64]
    ea = np.empty((9, E), dtype=np.float32)
    ea[0:3] = posT[:, ei0]
    ea[3:6] = posT[:, ei1]
    ea[6:9] = latP[0][:, batch_e]
    eb = np.empty((9, E), dtype=np.float32)
    eb[0:3] = latP[1][:, batch_e]
    eb[3:6] = latP[2][:, batch_e]
    eb[6:9] = edge_cell_shift.T

    if "edge" not in _cache:
        _cache["edge"] = _build_edge_prog()
    nc_edge = _cache["edge"]

    in_maps = []
    for c in range(NCORES):
        s = slice(c * ES, (c + 1) * ES)
        in_maps.append({
            "ea": np.ascontiguousarray(ea[:, s]),
            "eb": np.ascontiguousarray(eb[:, s]),
        })

    t0 = time.time()
    res1 = run_bass_kernel_spmd(nc_edge, in_maps, list(range(NCORES)), trace=TRACE)
    last_info["edge_wall_s"] = time.time() - t0
    last_info["edge_exec_ns"] = res1.exec_time_ns

    distances = np.concatenate([res1.results[c]["dist"] for c in range(NCORES)])
    recp = np.concatenate([res1.results[c]["recp"] for c in range(NCORES)], axis=1)  # [4, E]

    # ---- host staging for the triplet launch: re-shard rec planes ----
    r1 = recp[:, triplet_edge_index[0]]              # [4, T] f32
    r2 = recp[:, triplet_edge_index[1]]              # [4, T] f32

    if "tri" not in _cache:
        _cache["tri"] = _build_tri_prog()
    nc_tri = _cache["tri"]

    in_maps2 = []
    for c in range(NCORES):
        s = slice(c * TS, (c + 1) * TS)
        in_maps2.append({
            "r1": np.ascontiguousarray(r1[:, s]),
            "r2": np.ascontiguousarray(r2[:, s]),
        })

    t0 = time.time()
    res2 = run_bass_kernel_spmd(nc_tri, in_maps2, list(range(NCORES)), trace=TRACE)
    last_info["tri_wall_s"] = time.time() - t0
    last_info["tri_exec_ns"] = res2.exec_time_ns

    angles = np.concatenate([res2.results[c]["ang"] for c in range(NCORES)])
    return distances, angles
